# revision 4
# baseline (speedup 1.0000x reference)
"""Bayesian dense layer (per-sample reparameterized weights) on 8 TRN2 NeuronCores.

Computes out[b] = x[b] @ (W[b] * softplus(log_std) + mean) + bias for
B=512, IN=OUT=1024, data-parallel over the batch axis (64 rows per core).

v2.5: W/mean/S/x all uploaded as bf16 (HBM traffic halves vs f32; the kernel
is HBM-bound at ~344 GB/s/core).  i-blocks of 512 rows (4 per partition) so
each W tile DMA is a fully sequential 1 MiB with 8 KB per-partition lines.
Sample-term matmuls use N=1024 (single MM per (b, i-block, jj)).

Device algorithm per core (batch slice of BPC=64 rows):
  - layout: partition dim = i (contraction index), free dim = o; i-blocks of
    512 rows with i = blk*512 + 4p + jj
  - S = softplus(log_std) precomputed on host (bf16); x pre-transposed as
    xTh [IN, BPC] bf16
  - mean term: psum_mean[64, OUT] = xTh.T @ mean + ones.T @ bias, once at
    full PE width, copied to SBUF
  - per-sample term: W tiles [128, 4, OUT] bf16 stream 12-deep; the S
    multiply runs on DVE in 2x packed mode; per row b a [1, OUT] PSUM
    accumulator collects 8 bf16 matmuls; finished rows scatter back to
    partition b of an SBUF collector via small SBUF->SBUF DMAs
  - merge: one DVE add of the two [64, OUT] terms, one DMA to DRAM
"""

import os
import sys

for _p in ("/root/.axon_site", "/root/.axon_site/_ro/trn_rl_repo",
           "/root/.axon_site/_ro/pypackages"):
    if os.path.isdir(_p) and _p not in sys.path:
        sys.path.append(_p)

import numpy as np

import concourse.bass as bass
import concourse.mybir as mybir
import concourse.tile as tile
from concourse import bacc
from concourse.bass_utils import run_bass_kernel_spmd

B, IN, OUT = 512, 1024, 1024
NCORES = 8
BPC = B // NCORES  # batch rows per core
DBG = True         # small fp8e3 side-test (ACT convert + GP mixed mult)

_BUILT = {}


def build_bass(bpc=BPC, in_dim=IN, out_dim=OUT, wbufs=12, hbufs=5):
    """Build the per-core Bass module (all cores run the same program)."""
    key = (bpc, in_dim, out_dim, wbufs, hbufs)
    if key in _BUILT:
        return _BUILT[key]

    f32 = mybir.dt.float32
    bf16 = mybir.dt.bfloat16
    fp8e3 = mybir.dt.float8e3
    jjb = 4                       # i-rows per partition per block
    ibsz = 128 * jjb              # i-block size (512)
    nib = in_dim // ibsz          # i-blocks
    nch = max(1, out_dim // 512)  # output chunks per matmul (N<=512 ISA cap)
    chunk = out_dim // nch

    nc = bacc.Bacc("TRN2", target_bir_lowering=False, debug=False,
                   num_devices=NCORES)

    xTh = nc.dram_tensor("xTh", [in_dim, bpc], bf16, kind="ExternalInput").ap()
    W = nc.dram_tensor("W", [bpc, in_dim, out_dim], bf16,
                       kind="ExternalInput").ap()
    S = nc.dram_tensor("S", [in_dim, out_dim], bf16, kind="ExternalInput").ap()
    mean = nc.dram_tensor("mean", [in_dim, out_dim], bf16,
                          kind="ExternalInput").ap()
    bias = nc.dram_tensor("bias", [1, out_dim], f32, kind="ExternalInput").ap()
    out = nc.dram_tensor("out", [bpc, out_dim], f32,
                         kind="ExternalOutput").ap()
    if DBG:
        w8dbg = nc.dram_tensor("w8dbg", [128, 256], fp8e3,
                               kind="ExternalInput").ap()
        dbg = nc.dram_tensor("dbg", [2, 128, 256], bf16,
                             kind="ExternalOutput").ap()

    with tile.TileContext(nc) as tc:
        with (
            tc.tile_pool(name="singles", bufs=1) as singles,
            tc.tile_pool(name="wpool", bufs=wbufs) as wpool,
            tc.tile_pool(name="hpool", bufs=hbufs) as hpool,
            tc.tile_pool(name="opool", bufs=2) as opool,
            tc.tile_pool(name="psum", bufs=1, space="PSUM") as psum,
            tc.tile_pool(name="psrow", bufs=3, space="PSUM") as psrow,
        ):
            xTh_sb = singles.tile([128, nib, jjb, bpc], bf16)
            nc.sync.dma_start(
                out=xTh_sb,
                in_=xTh.rearrange("(ib p jj) b -> p ib jj b", p=128, jj=jjb))
            S_sb = singles.tile([128, nib, jjb, out_dim], bf16)
            nc.sync.dma_start(
                out=S_sb,
                in_=S.rearrange("(ib p jj) o -> p ib jj o", p=128, jj=jjb))
            bias_sb = singles.tile([1, out_dim], f32)
            nc.sync.dma_start(out=bias_sb, in_=bias)
            ones = singles.tile([1, bpc], f32)
            nc.vector.memset(ones, 1.0)

            # ── small fp8e3 mechanism test (independent of the main path) ──
            if DBG:
                t8 = singles.tile([128, 256], fp8e3)
                nc.sync.dma_start(out=t8, in_=w8dbg)
                tb = singles.tile([128, 256], bf16)
                nc.scalar.copy(tb, t8)          # ACT fp8e3 -> bf16 convert
                td = singles.tile([128, 256], bf16)
                nc.vector.tensor_mul(td, tb, S_sb[:, 0, 0, :256])
                tg = singles.tile([128, 256], bf16)
                nc.gpsimd.tensor_mul(tg, t8, S_sb[:, 0, 0, :256])
                nc.sync.dma_start(out=dbg[0], in_=td)
                nc.sync.dma_start(out=dbg[1], in_=tg)

            # ── mean term at full PE width: mb_sb = xTh.T @ mean + bias ──
            acc_m = psum.tile([bpc, out_dim], f32)
            for ib in range(nib):
                m_t = wpool.tile([128, jjb, out_dim], bf16, tag="w",
                                 name=f"m_t{ib}")
                nc.sync.dma_start(
                    out=m_t,
                    in_=mean[ib * ibsz:(ib + 1) * ibsz, :]
                    .rearrange("(p jj) o -> p jj o", jj=jjb))
                for jj in range(jjb):
                    for n in range(nch):
                        nc.tensor.matmul(
                            acc_m[:, n * chunk:(n + 1) * chunk],
                            xTh_sb[:, ib, jj, :],
                            m_t[:, jj, n * chunk:(n + 1) * chunk],
                            start=(ib == 0 and jj == 0), stop=False,
                            skip_group_check=True)
            for n in range(nch):
                nc.tensor.matmul(
                    acc_m[:, n * chunk:(n + 1) * chunk],
                    ones,
                    bias_sb[:, n * chunk:(n + 1) * chunk],
                    start=False, stop=True, skip_group_check=True)
            mb_sb = singles.tile([bpc, out_dim], f32)
            nc.scalar.copy(mb_sb, acc_m)

            # ── per-sample term, collected per row into wt_sb ──
            wt_sb = singles.tile([bpc, out_dim], f32)
            for b in range(bpc):
                acc = psrow.tile([1, out_dim], f32, tag="acc", name=f"acc{b}")
                for ib in range(nib):
                    w_t = wpool.tile([128, jjb, out_dim], bf16, tag="w",
                                     name=f"w_t{b}_{ib}")
                    nc.sync.dma_start(
                        out=w_t,
                        in_=W[b, ib * ibsz:(ib + 1) * ibsz, :]
                        .rearrange("(p jj) o -> p jj o", jj=jjb))
                    w_h = hpool.tile([128, jjb, out_dim], bf16, tag="wh",
                                     name=f"w_h{b}_{ib}")
                    # bf16 x bf16 -> bf16 runs in DVE 2x packed mode
                    nc.vector.tensor_mul(w_h, w_t, S_sb[:, ib])
                    for jj in range(jjb):
                        for n in range(nch):
                            nc.tensor.matmul(
                                acc[:, n * chunk:(n + 1) * chunk],
                                xTh_sb[:, ib, jj, b:b + 1],
                                w_h[:, jj, n * chunk:(n + 1) * chunk],
                                start=(ib == 0 and jj == 0),
                                stop=(ib == nib - 1 and jj == jjb - 1),
                                skip_group_check=True)
                row = opool.tile([1, out_dim], f32, tag="row",
                                 name=f"row{b}")
                nc.scalar.copy(row, acc)
                nc.sync.dma_start(out=wt_sb[b:b + 1, :], in_=row)

            # ── merge and write out ──
            nc.vector.tensor_add(wt_sb, wt_sb, mb_sb)
            nc.sync.dma_start(out=out, in_=wt_sb)

    nc.finalize()
    _BUILT[key] = nc
    return nc


def _softplus(x):
    return np.logaddexp(0.0, x.astype(np.float32)).astype(np.float32)


def _run(x, W, mean, log_std, bias, **kwargs):
    import ml_dtypes
    bf16 = ml_dtypes.bfloat16
    x = np.ascontiguousarray(x, dtype=np.float32)
    Wh = np.ascontiguousarray(W, dtype=np.float32).astype(bf16)
    mean_h = np.ascontiguousarray(mean, dtype=np.float32).astype(bf16)
    bias2 = np.ascontiguousarray(bias, dtype=np.float32).reshape(1, OUT)
    S = _softplus(log_std).astype(bf16)

    nc = build_bass()
    rng = np.random.RandomState(0)
    w8 = (rng.randn(128, 256).astype(np.float32) * 1.6).astype(
        ml_dtypes.float8_e3m4)
    in_maps = []
    for c in range(NCORES):
        sl = slice(c * BPC, (c + 1) * BPC)
        m = {
            "xTh": np.ascontiguousarray(x[sl].T).astype(bf16),
            "W": Wh[sl],
            "S": S,
            "mean": mean_h,
            "bias": bias2,
        }
        if DBG:
            m["w8dbg"] = w8
        in_maps.append(m)
    res = run_bass_kernel_spmd(nc, in_maps, core_ids=list(range(NCORES)),
                               **kwargs)
    if DBG:
        d = res.results[0]["dbg"]
        sref = S.astype(np.float32)[0:512:4, :256]  # S_sb[:,0,0,:256] = i=4p
        ref = (w8.astype(np.float32) * sref).astype(bf16).astype(np.float32)
        td = np.asarray(d[0]).astype(np.float32)
        tg = np.asarray(d[1]).astype(np.float32)
        print("DBG act-convert+dve-mult maxerr:",
              np.abs(td - ref).max(),
              "| gpsimd fp8 mult maxerr:", np.abs(tg - ref).max())
    out = np.concatenate([res.results[c]["out"] for c in range(NCORES)],
                         axis=0)
    return out, res


def kernel(x, W, mean, log_std, bias):
    return _run(x, W, mean, log_std, bias)[0]


# revision 8
# speedup vs baseline: 1.0894x; 1.0894x over previous
"""Bayesian dense layer (per-sample reparameterized weights) on 8 TRN2 NeuronCores.

Computes out[b] = x[b] @ (W[b] * softplus(log_std) + mean) + bias for
B=512, IN=OUT=1024, data-parallel over the batch axis (64 rows per core).

v3: W is uploaded per-tile in a mix of bf16 and fp8e3 (e3m4, pre-scaled x32
on host; the matching stationary x column is pre-divided by 32 on device, an
exact exponent shift).  This cuts HBM W traffic from 134 MB (all-bf16) to
~90 MB per core.  fp8 tiles reach the bf16 matmul path via two channels:
ACT copy-convert (then DVE multiply) or a direct GpSimd fp8xbf16 multiply.
Per-core engine budget at the ~290 us target: DMA ~95 MB, DVE ~220 us,
ACT ~280 us, GpSimd ~250 us, PE ~280 us -- all co-saturated.

Device algorithm per core (batch slice of BPC=64 rows):
  - layout: partition dim = i, free dim = o; i-blocks of 512 rows with
    i = blk*512 + 4p + jj; each W tile DMA is fully sequential (1 MiB bf16 /
    512 KiB fp8)
  - S = softplus(log_std) precomputed on host (bf16)
  - mean term: psum_mean[64, OUT] = xTh.T @ mean + ones.T @ bias once at
    full PE width
  - per-sample term: per row b a [1, OUT] PSUM accumulator collects 16 bf16
    matmuls (N=512); finished rows scatter to an SBUF collector via small
    SBUF->SBUF DMAs; channel pattern interleaves BF/ACT/GP tiles evenly
  - merge: one DVE add of the two [64, OUT] terms, one DMA to DRAM
"""

import os
import sys

for _p in ("/root/.axon_site", "/root/.axon_site/_ro/trn_rl_repo",
           "/root/.axon_site/_ro/pypackages"):
    if os.path.isdir(_p) and _p not in sys.path:
        sys.path.append(_p)

import numpy as np

import concourse.bass as bass
import concourse.mybir as mybir
import concourse.tile as tile
from concourse import bacc
from concourse.bass_utils import run_bass_kernel_spmd

B, IN, OUT = 512, 1024, 1024
NCORES = 8
BPC = B // NCORES  # batch rows per core
F8SCALE = 32.0     # host premultiplies fp8 tiles by this; x column divided

# channel pattern over flat tile index t = b*nib + ib (cycle of 32):
# B = bf16 upload + DVE mult; A = fp8 upload + ACT convert + DVE mult;
# G = fp8 upload + GpSimd direct mult
PAT = "ABGABAGBABAGABABGABAGABABGABAGBA"

_BUILT = {}


def _channels(ntiles):
    return [PAT[t % len(PAT)] for t in range(ntiles)]


def build_bass(bpc=BPC, in_dim=IN, out_dim=OUT):
    key = (bpc, in_dim, out_dim)
    if key in _BUILT:
        return _BUILT[key]

    f32 = mybir.dt.float32
    bf16 = mybir.dt.bfloat16
    fp8e3 = mybir.dt.float8e3
    jjb = 4
    ibsz = 128 * jjb              # 512
    nib = in_dim // ibsz          # 2
    nch = max(1, out_dim // 512)
    chunk = out_dim // nch

    ch = _channels(bpc * nib)
    nBF = sum(c == "B" for c in ch)
    nF8 = len(ch) - nBF

    nc = bacc.Bacc("TRN2", target_bir_lowering=False, debug=False,
                   num_devices=NCORES)

    xTh = nc.dram_tensor("xTh", [in_dim, bpc], bf16, kind="ExternalInput").ap()
    W_bf = nc.dram_tensor("W_bf", [max(nBF, 1), ibsz, out_dim], bf16,
                          kind="ExternalInput").ap()
    W_f8 = nc.dram_tensor("W_f8", [max(nF8, 1), ibsz, out_dim], fp8e3,
                          kind="ExternalInput").ap()
    S = nc.dram_tensor("S", [in_dim, out_dim], bf16, kind="ExternalInput").ap()
    mean = nc.dram_tensor("mean", [in_dim, out_dim], bf16,
                          kind="ExternalInput").ap()
    bias = nc.dram_tensor("bias", [1, out_dim], f32, kind="ExternalInput").ap()
    out = nc.dram_tensor("out", [bpc, out_dim], f32,
                         kind="ExternalOutput").ap()

    with tile.TileContext(nc) as tc:
        with (
            tc.tile_pool(name="singles", bufs=1) as singles,
            tc.tile_pool(name="wbf", bufs=6) as wbf,
            tc.tile_pool(name="wf8", bufs=10) as wf8,
            tc.tile_pool(name="cpool", bufs=4) as cpool,
            tc.tile_pool(name="hpool", bufs=5) as hpool,
            tc.tile_pool(name="opool", bufs=2) as opool,
            tc.tile_pool(name="psum", bufs=1, space="PSUM") as psum,
            tc.tile_pool(name="psrow", bufs=3, space="PSUM") as psrow,
        ):
            xTh_sb = singles.tile([128, nib, jjb, bpc], bf16)
            nc.sync.dma_start(
                out=xTh_sb,
                in_=xTh.rearrange("(ib p jj) b -> p ib jj b", p=128, jj=jjb))
            xT32_sb = singles.tile([128, nib, jjb, bpc], bf16)
            nc.vector.tensor_scalar_mul(xT32_sb, xTh_sb, 1.0 / F8SCALE)
            S_sb = singles.tile([128, nib, jjb, out_dim], bf16)
            nc.sync.dma_start(
                out=S_sb,
                in_=S.rearrange("(ib p jj) o -> p ib jj o", p=128, jj=jjb))
            bias_sb = singles.tile([1, out_dim], f32)
            nc.sync.dma_start(out=bias_sb, in_=bias)
            ones = singles.tile([1, bpc], f32)
            nc.vector.memset(ones, 1.0)

            # ── mean term at full PE width: mb_sb = xTh.T @ mean + bias ──
            acc_m = psum.tile([bpc, out_dim], f32)
            for ib in range(nib):
                m_t = wbf.tile([128, jjb, out_dim], bf16, tag="w",
                               name=f"m_t{ib}")
                nc.sync.dma_start(
                    out=m_t,
                    in_=mean[ib * ibsz:(ib + 1) * ibsz, :]
                    .rearrange("(p jj) o -> p jj o", jj=jjb))
                for jj in range(jjb):
                    for n in range(nch):
                        nc.tensor.matmul(
                            acc_m[:, n * chunk:(n + 1) * chunk],
                            xTh_sb[:, ib, jj, :],
                            m_t[:, jj, n * chunk:(n + 1) * chunk],
                            start=(ib == 0 and jj == 0), stop=False,
                            skip_group_check=True)
            for n in range(nch):
                nc.tensor.matmul(
                    acc_m[:, n * chunk:(n + 1) * chunk],
                    ones,
                    bias_sb[:, n * chunk:(n + 1) * chunk],
                    start=False, stop=True, skip_group_check=True)
            mb_sb = singles.tile([bpc, out_dim], f32)
            nc.scalar.copy(mb_sb, acc_m)

            # ── per-sample term ──
            wt_sb = singles.tile([bpc, out_dim], f32)
            kB = kF = 0
            for b in range(bpc):
                acc = psrow.tile([1, out_dim], f32, tag="acc", name=f"acc{b}")
                for ib in range(nib):
                    t = b * nib + ib
                    c = ch[t]
                    if c == "B":
                        w_t = wbf.tile([128, jjb, out_dim], bf16, tag="w",
                                       name=f"w_t{t}")
                        nc.sync.dma_start(
                            out=w_t,
                            in_=W_bf[kB].rearrange("(p jj) o -> p jj o",
                                                   jj=jjb))
                        kB += 1
                        w_h = hpool.tile([128, jjb, out_dim], bf16, tag="wh",
                                         name=f"w_h{t}")
                        nc.vector.tensor_mul(w_h, w_t, S_sb[:, ib])
                        stat = xTh_sb
                    else:
                        w8 = wf8.tile([128, jjb, out_dim], fp8e3, tag="w8",
                                      name=f"w8_{t}")
                        nc.sync.dma_start(
                            out=w8,
                            in_=W_f8[kF].rearrange("(p jj) o -> p jj o",
                                                   jj=jjb))
                        kF += 1
                        w_h = hpool.tile([128, jjb, out_dim], bf16, tag="wh",
                                         name=f"w_h{t}")
                        if c == "A":
                            w_c = cpool.tile([128, jjb, out_dim], bf16,
                                             tag="wc", name=f"w_c{t}")
                            nc.scalar.copy(w_c, w8)
                            nc.vector.tensor_mul(w_h, w_c, S_sb[:, ib])
                        else:  # G
                            nc.gpsimd.tensor_mul(w_h, w8, S_sb[:, ib])
                        stat = xT32_sb
                    for jj in range(jjb):
                        for n in range(nch):
                            nc.tensor.matmul(
                                acc[:, n * chunk:(n + 1) * chunk],
                                stat[:, ib, jj, b:b + 1],
                                w_h[:, jj, n * chunk:(n + 1) * chunk],
                                start=(ib == 0 and jj == 0),
                                stop=(ib == nib - 1 and jj == jjb - 1),
                                skip_group_check=True)
                row = opool.tile([1, out_dim], f32, tag="row",
                                 name=f"row{b}")
                # split PSUM drains between ACT and DVE to balance load
                if b % 2 == 0:
                    nc.scalar.copy(row, acc)
                else:
                    nc.vector.tensor_copy(row, acc)
                nc.scalar.dma_start(out=wt_sb[b:b + 1, :], in_=row)

            # ── merge and write out ──
            nc.vector.tensor_add(wt_sb, wt_sb, mb_sb)
            nc.scalar.dma_start(out=out, in_=wt_sb)

    nc.finalize()
    _BUILT[key] = nc
    return nc


def _softplus(x):
    return np.logaddexp(0.0, x.astype(np.float32)).astype(np.float32)


def _run(x, W, mean, log_std, bias, **kwargs):
    import ml_dtypes
    bf16 = ml_dtypes.bfloat16
    fp8 = ml_dtypes.float8_e3m4
    jjb = 4
    ibsz = 128 * jjb
    nib = IN // ibsz
    ch = _channels(BPC * nib)

    x = np.ascontiguousarray(x, dtype=np.float32)
    W = np.ascontiguousarray(W, dtype=np.float32)
    mean_h = np.ascontiguousarray(mean, dtype=np.float32).astype(bf16)
    bias2 = np.ascontiguousarray(bias, dtype=np.float32).reshape(1, OUT)
    S = _softplus(log_std).astype(bf16)

    nc = build_bass()
    in_maps = []
    for cix in range(NCORES):
        sl = slice(cix * BPC, (cix + 1) * BPC)
        Wc = W[sl]  # [BPC, IN, OUT] f32
        tiles = Wc.reshape(BPC * nib, ibsz, OUT)
        bsel = np.array([c == "B" for c in ch])
        W_bf = tiles[bsel].astype(bf16)
        W_f8 = (tiles[~bsel] * F8SCALE).astype(fp8)
        in_maps.append({
            "xTh": np.ascontiguousarray(x[sl].T).astype(bf16),
            "W_bf": W_bf,
            "W_f8": W_f8,
            "S": S,
            "mean": mean_h,
            "bias": bias2,
        })
    res = run_bass_kernel_spmd(nc, in_maps, core_ids=list(range(NCORES)),
                               **kwargs)
    out = np.concatenate([res.results[c]["out"] for c in range(NCORES)],
                         axis=0)
    return out, res


def kernel(x, W, mean, log_std, bias):
    return _run(x, W, mean, log_std, bias)[0]


# revision 9
# speedup vs baseline: 1.4146x; 1.2985x over previous
"""Bayesian dense layer (per-sample reparameterized weights) on 8 TRN2 NeuronCores.

Computes out[b] = x[b] @ (W[b] * softplus(log_std) + mean) + bias for
B=512, IN=OUT=1024, data-parallel over the batch axis (64 rows per core).

v3b: W is uploaded per-tile in a mix of bf16 and fp8e3 (e3m4, pre-scaled x32
on host; the matching stationary x column is pre-divided by 32 on device, an
exact exponent shift).  fp8 tiles reach the bf16 matmul path via ACT
copy-convert (then DVE multiply) or a direct GpSimd fp8xbf16 multiply.
Batch rows are processed in groups of 4 with column-tiled matmuls
(tile_position=(0,32j)): the four rows' N=512 matmuls run concurrently in
disjoint 32-column groups of the PE array, and their PSUM accumulators live
at partitions {0,32,64,96} of one [128, OUT] psum tile, so a group drains
with a single ACT copy + a single 4-row scatter DMA.

Per-core budget at the ~275 us target: HBM ~91 MB @ ~330 GB/s, ACT ~265 us
(converts + drains), GpSimd ~266 us, DVE ~220 us, PE no longer binding.
"""

import os
import sys

for _p in ("/root/.axon_site", "/root/.axon_site/_ro/trn_rl_repo",
           "/root/.axon_site/_ro/pypackages"):
    if os.path.isdir(_p) and _p not in sys.path:
        sys.path.append(_p)

import numpy as np

import concourse.bass as bass
import concourse.mybir as mybir
import concourse.tile as tile
from concourse import bacc
from concourse.bass_utils import run_bass_kernel_spmd

B, IN, OUT = 512, 1024, 1024
NCORES = 8
BPC = B // NCORES  # batch rows per core
F8SCALE = 32.0     # host premultiplies fp8 tiles by this; x column divided
GRP = 4            # batch rows per column-tiled PE group

_BUILT = {}


def _channels(nsets):
    """Channel per tile, consumed in (set, j) order; one set = 4 tiles of
    consecutive rows b=4g+j at one i-block.  GpSimd is unusable here (any GP
    op blocks concurrent DVE tensor_tensor on the shared SBUF port), so all
    multiplies run on DVE; ACT converts as many fp8 tiles as it can absorb."""
    ch = []
    for s in range(nsets):
        ch += (["A", "B", "A", "A"] if s % 3 == 0 else ["A", "B", "A", "B"])
    return ch


def build_bass(bpc=BPC, in_dim=IN, out_dim=OUT):
    key = (bpc, in_dim, out_dim)
    if key in _BUILT:
        return _BUILT[key]

    f32 = mybir.dt.float32
    bf16 = mybir.dt.bfloat16
    fp8e3 = mybir.dt.float8e3
    jjb = 4
    ibsz = 128 * jjb              # 512
    nib = in_dim // ibsz          # 2
    nch = max(1, out_dim // 512)
    chunk = out_dim // nch
    ngrp = bpc // GRP             # 16 row groups
    nset = ngrp * nib             # 32 tile sets of 4

    ch = _channels(nset)
    nBF = sum(c == "B" for c in ch)
    nF8 = len(ch) - nBF

    nc = bacc.Bacc("TRN2", target_bir_lowering=False, debug=False,
                   num_devices=NCORES)

    xTh = nc.dram_tensor("xTh", [in_dim, bpc], bf16, kind="ExternalInput").ap()
    W_bf = nc.dram_tensor("W_bf", [max(nBF, 1), ibsz, out_dim], bf16,
                          kind="ExternalInput").ap()
    W_f8 = nc.dram_tensor("W_f8", [max(nF8, 1), ibsz, out_dim], fp8e3,
                          kind="ExternalInput").ap()
    S = nc.dram_tensor("S", [in_dim, out_dim], bf16, kind="ExternalInput").ap()
    mean = nc.dram_tensor("mean", [in_dim, out_dim], bf16,
                          kind="ExternalInput").ap()
    bias = nc.dram_tensor("bias", [1, out_dim], f32, kind="ExternalInput").ap()
    out = nc.dram_tensor("out", [bpc, out_dim], f32,
                         kind="ExternalOutput").ap()

    with tile.TileContext(nc) as tc:
        with (
            tc.tile_pool(name="singles", bufs=1) as singles,
            tc.tile_pool(name="wbf", bufs=5) as wbf,
            tc.tile_pool(name="wf8", bufs=8) as wf8,
            tc.tile_pool(name="cpool", bufs=3) as cpool,
            tc.tile_pool(name="hpool", bufs=8) as hpool,
            tc.tile_pool(name="opool", bufs=2) as opool,
            tc.tile_pool(name="psum", bufs=1, space="PSUM") as psum,
            tc.tile_pool(name="psg", bufs=2, space="PSUM") as psg,
        ):
            xTh_sb = singles.tile([128, nib, jjb, bpc], bf16)
            nc.sync.dma_start(
                out=xTh_sb,
                in_=xTh.rearrange("(ib p jj) b -> p ib jj b", p=128, jj=jjb))
            xT32_sb = singles.tile([128, nib, jjb, bpc], bf16)
            nc.vector.tensor_scalar_mul(xT32_sb, xTh_sb, 1.0 / F8SCALE)
            S_sb = singles.tile([128, nib, jjb, out_dim], bf16)
            nc.sync.dma_start(
                out=S_sb,
                in_=S.rearrange("(ib p jj) o -> p ib jj o", p=128, jj=jjb))
            bias_sb = singles.tile([1, out_dim], f32)
            nc.sync.dma_start(out=bias_sb, in_=bias)
            ones = singles.tile([1, bpc], f32)
            nc.vector.memset(ones, 1.0)

            # ── mean term at full PE width: mb_sb = xTh.T @ mean + bias ──
            acc_m = psum.tile([bpc, out_dim], f32)
            for ib in range(nib):
                m_t = wbf.tile([128, jjb, out_dim], bf16, tag="w",
                               name=f"m_t{ib}")
                nc.sync.dma_start(
                    out=m_t,
                    in_=mean[ib * ibsz:(ib + 1) * ibsz, :]
                    .rearrange("(p jj) o -> p jj o", jj=jjb))
                for jj in range(jjb):
                    for n in range(nch):
                        nc.tensor.matmul(
                            acc_m[:, n * chunk:(n + 1) * chunk],
                            xTh_sb[:, ib, jj, :],
                            m_t[:, jj, n * chunk:(n + 1) * chunk],
                            start=(ib == 0 and jj == 0), stop=False,
                            skip_group_check=True)
            for n in range(nch):
                nc.tensor.matmul(
                    acc_m[:, n * chunk:(n + 1) * chunk],
                    ones,
                    bias_sb[:, n * chunk:(n + 1) * chunk],
                    start=False, stop=True, skip_group_check=True)
            mb_sb = singles.tile([bpc, out_dim], f32)
            nc.scalar.copy(mb_sb, acc_m)

            # ── per-sample term, 4 rows per column-tiled PE group ──
            wt_sb = singles.tile([bpc, out_dim], f32)
            kB = kF = 0
            t = 0
            for g in range(ngrp):
                acc4 = psg.tile([128, out_dim], f32, tag="acc",
                                name=f"acc{g}")
                stats = [None] * (nib * GRP)
                whs = [None] * (nib * GRP)
                for ib in range(nib):
                    for j in range(GRP):
                        c = ch[t]
                        t += 1
                        if c == "B":
                            w_t = wbf.tile([128, jjb, out_dim], bf16, tag="w",
                                           name=f"w_t{g}_{ib}_{j}")
                            nc.sync.dma_start(
                                out=w_t,
                                in_=W_bf[kB].rearrange(
                                    "(p jj) o -> p jj o", jj=jjb))
                            kB += 1
                            w_h = hpool.tile([128, jjb, out_dim], bf16,
                                             tag="wh", name=f"w_h{g}_{ib}_{j}")
                            nc.vector.tensor_mul(w_h, w_t, S_sb[:, ib])
                            stat = xTh_sb
                        else:
                            w8 = wf8.tile([128, jjb, out_dim], fp8e3,
                                          tag="w8", name=f"w8_{g}_{ib}_{j}")
                            nc.sync.dma_start(
                                out=w8,
                                in_=W_f8[kF].rearrange(
                                    "(p jj) o -> p jj o", jj=jjb))
                            kF += 1
                            w_h = hpool.tile([128, jjb, out_dim], bf16,
                                             tag="wh", name=f"w_h{g}_{ib}_{j}")
                            if c == "A":
                                w_c = cpool.tile([128, jjb, out_dim], bf16,
                                                 tag="wc",
                                                 name=f"w_c{g}_{ib}_{j}")
                                nc.scalar.copy(w_c, w8)
                                nc.vector.tensor_mul(w_h, w_c, S_sb[:, ib])
                            else:  # G
                                nc.gpsimd.tensor_mul(w_h, w8, S_sb[:, ib])
                            stat = xT32_sb
                        stats[ib * GRP + j] = stat
                        whs[ib * GRP + j] = w_h
                    # issue the 4 rows' matmuls j-innermost so they run
                    # concurrently in disjoint 32-column groups
                    for jj in range(jjb):
                        for n in range(nch):
                            for j in range(GRP):
                                b = g * GRP + j
                                nc.tensor.matmul(
                                    acc4[32 * j:32 * j + 1,
                                         n * chunk:(n + 1) * chunk],
                                    stats[ib * GRP + j][:, ib, jj, b:b + 1],
                                    whs[ib * GRP + j][:, jj,
                                                      n * chunk:(n + 1) * chunk],
                                    start=(ib == 0 and jj == 0),
                                    stop=(ib == nib - 1 and jj == jjb - 1),
                                    tile_position=(0, 32 * j),
                                    skip_group_check=True)
                # drain all 4 rows with one ACT copy + one scatter DMA
                col4 = opool.tile([128, out_dim], f32, tag="col",
                                  name=f"col{g}")
                nc.scalar.copy(col4, acc4)
                nc.scalar.dma_start(
                    out=wt_sb[g * GRP:(g + 1) * GRP, :],
                    in_=col4.rearrange("(j q) o -> j q o", j=GRP)[:, 0])

            # ── merge and write out ──
            nc.vector.tensor_add(wt_sb, wt_sb, mb_sb)
            nc.scalar.dma_start(out=out, in_=wt_sb)

    nc.finalize()
    _BUILT[key] = nc
    return nc


def _softplus(x):
    return np.logaddexp(0.0, x.astype(np.float32)).astype(np.float32)


def _run(x, W, mean, log_std, bias, **kwargs):
    import ml_dtypes
    bf16 = ml_dtypes.bfloat16
    fp8 = ml_dtypes.float8_e3m4
    jjb = 4
    ibsz = 128 * jjb
    nib = IN // ibsz
    ngrp = BPC // GRP
    ch = _channels(ngrp * nib)

    x = np.ascontiguousarray(x, dtype=np.float32)
    W = np.ascontiguousarray(W, dtype=np.float32)
    mean_h = np.ascontiguousarray(mean, dtype=np.float32).astype(bf16)
    bias2 = np.ascontiguousarray(bias, dtype=np.float32).reshape(1, OUT)
    S = _softplus(log_std).astype(bf16)

    nc = build_bass()
    in_maps = []
    for cix in range(NCORES):
        sl = slice(cix * BPC, (cix + 1) * BPC)
        Wc = W[sl]  # [BPC, IN, OUT] f32
        bf_tiles, f8_tiles = [], []
        ti = 0
        for g in range(ngrp):
            for ib in range(nib):
                for j in range(GRP):
                    tile_np = Wc[g * GRP + j, ib * ibsz:(ib + 1) * ibsz, :]
                    if ch[ti] == "B":
                        bf_tiles.append(tile_np.astype(bf16))
                    else:
                        f8_tiles.append((tile_np * F8SCALE).astype(fp8))
                    ti += 1
        in_maps.append({
            "xTh": np.ascontiguousarray(x[sl].T).astype(bf16),
            "W_bf": np.stack(bf_tiles),
            "W_f8": np.stack(f8_tiles),
            "S": S,
            "mean": mean_h,
            "bias": bias2,
        })
    res = run_bass_kernel_spmd(nc, in_maps, core_ids=list(range(NCORES)),
                               **kwargs)
    out = np.concatenate([res.results[c]["out"] for c in range(NCORES)],
                         axis=0)
    return out, res


def kernel(x, W, mean, log_std, bias):
    return _run(x, W, mean, log_std, bias)[0]


# revision 14
# speedup vs baseline: 1.6326x; 1.1541x over previous
"""Bayesian dense layer (per-sample reparameterized weights) on 8 TRN2 NeuronCores.

Computes out[b] = x[b] @ (W[b] * softplus(log_std) + mean) + bias for
B=512, IN=OUT=1024, data-parallel over the batch axis (64 rows per core).

v3b: W is uploaded per-tile in a mix of bf16 and fp8e3 (e3m4, pre-scaled x32
on host; the matching stationary x column is pre-divided by 32 on device, an
exact exponent shift).  fp8 tiles reach the bf16 matmul path via ACT
copy-convert (then DVE multiply) or a direct GpSimd fp8xbf16 multiply.
Batch rows are processed in groups of 4 with column-tiled matmuls
(tile_position=(0,32j)): the four rows' N=512 matmuls run concurrently in
disjoint 32-column groups of the PE array, and their PSUM accumulators live
at partitions {0,32,64,96} of one [128, OUT] psum tile, so a group drains
with a single ACT copy + a single 4-row scatter DMA.

Per-core budget at the ~275 us target: HBM ~91 MB @ ~330 GB/s, ACT ~265 us
(converts + drains), GpSimd ~266 us, DVE ~220 us, PE no longer binding.
"""

import os
import sys

for _p in ("/root/.axon_site", "/root/.axon_site/_ro/trn_rl_repo",
           "/root/.axon_site/_ro/pypackages"):
    if os.path.isdir(_p) and _p not in sys.path:
        sys.path.append(_p)

import numpy as np

import concourse.bass as bass
import concourse.mybir as mybir
import concourse.tile as tile
from concourse import bacc
from concourse.bass_utils import run_bass_kernel_spmd

B, IN, OUT = 512, 1024, 1024
NCORES = 8
BPC = B // NCORES  # batch rows per core
F8SCALE = 32.0     # host premultiplies fp8 tiles by this; x column divided
GRP = 4            # batch rows per column-tiled PE group

_BUILT = {}


def _channels(nsets):
    """Channel per tile, consumed in (set, j) order; one set = 4 tiles of
    consecutive rows b=4g+j at one i-block.  GpSimd is unusable here (any GP
    op blocks concurrent DVE tensor_tensor on the shared SBUF port), so all
    multiplies run on DVE; ACT converts as many fp8 tiles as it can absorb."""
    ch = []
    for s in range(nsets):
        ch += (["A", "B", "A", "A"] if s % 4 == 1 else ["A", "B", "A", "B"])
    return ch


def build_bass(bpc=BPC, in_dim=IN, out_dim=OUT):
    key = (bpc, in_dim, out_dim)
    if key in _BUILT:
        return _BUILT[key]

    f32 = mybir.dt.float32
    bf16 = mybir.dt.bfloat16
    fp8e3 = mybir.dt.float8e3
    jjb = 4
    ibsz = 128 * jjb              # 512
    nib = in_dim // ibsz          # 2
    nch = max(1, out_dim // 512)
    chunk = out_dim // nch
    ngrp = bpc // GRP             # 16 row groups
    nset = ngrp * nib             # 32 tile sets of 4

    ch = _channels(nset)
    nBF = sum(c == "B" for c in ch)
    nF8 = len(ch) - nBF

    nc = bacc.Bacc("TRN2", target_bir_lowering=False, debug=False,
                   num_devices=NCORES)

    xTh = nc.dram_tensor("xTh", [in_dim, bpc], bf16, kind="ExternalInput").ap()
    W_bf = nc.dram_tensor("W_bf", [max(nBF, 1), ibsz, out_dim], bf16,
                          kind="ExternalInput").ap()
    W_f8 = nc.dram_tensor("W_f8", [max(nF8, 1), ibsz, out_dim], fp8e3,
                          kind="ExternalInput").ap()
    S = nc.dram_tensor("S", [in_dim, out_dim], bf16, kind="ExternalInput").ap()
    mean = nc.dram_tensor("mean", [in_dim, out_dim], bf16,
                          kind="ExternalInput").ap()
    bias = nc.dram_tensor("bias", [1, out_dim], f32, kind="ExternalInput").ap()
    out = nc.dram_tensor("out", [bpc, out_dim], f32,
                         kind="ExternalOutput").ap()

    with tile.TileContext(nc) as tc:
        with (
            tc.tile_pool(name="singles", bufs=1) as singles,
            tc.tile_pool(name="wbf", bufs=5) as wbf,
            tc.tile_pool(name="wf8", bufs=10) as wf8,
            tc.tile_pool(name="cpool", bufs=3) as cpool,
            tc.tile_pool(name="hpool", bufs=8) as hpool,
            tc.tile_pool(name="opool", bufs=2) as opool,
            tc.tile_pool(name="psum", bufs=1, space="PSUM") as psum,
            tc.tile_pool(name="psg", bufs=2, space="PSUM") as psg,
        ):
            xTh_sb = singles.tile([128, nib, jjb, bpc], bf16)
            nc.sync.dma_start(
                out=xTh_sb,
                in_=xTh.rearrange("(ib p jj) b -> p ib jj b", p=128, jj=jjb))
            xT32_sb = singles.tile([128, nib, jjb, bpc], bf16)
            nc.vector.tensor_scalar_mul(xT32_sb, xTh_sb, 1.0 / F8SCALE)
            S_sb = singles.tile([128, nib, jjb, out_dim], bf16)
            nc.sync.dma_start(
                out=S_sb,
                in_=S.rearrange("(ib p jj) o -> p ib jj o", p=128, jj=jjb))
            bias_sb = singles.tile([1, out_dim], f32)
            nc.sync.dma_start(out=bias_sb, in_=bias)
            ones = singles.tile([1, bpc], f32)
            nc.vector.memset(ones, 1.0)

            # ── mean term at full PE width: mb_sb = xTh.T @ mean + bias ──
            acc_m = psum.tile([bpc, out_dim], f32)
            for ib in range(nib):
                m_t = wbf.tile([128, jjb, out_dim], bf16, tag="w",
                               name=f"m_t{ib}")
                nc.sync.dma_start(
                    out=m_t,
                    in_=mean[ib * ibsz:(ib + 1) * ibsz, :]
                    .rearrange("(p jj) o -> p jj o", jj=jjb))
                for jj in range(jjb):
                    for n in range(nch):
                        nc.tensor.matmul(
                            acc_m[:, n * chunk:(n + 1) * chunk],
                            xTh_sb[:, ib, jj, :],
                            m_t[:, jj, n * chunk:(n + 1) * chunk],
                            start=(ib == 0 and jj == 0), stop=False,
                            skip_group_check=True)
            for n in range(nch):
                nc.tensor.matmul(
                    acc_m[:, n * chunk:(n + 1) * chunk],
                    ones,
                    bias_sb[:, n * chunk:(n + 1) * chunk],
                    start=False, stop=True, skip_group_check=True)
            mb_sb = singles.tile([bpc, out_dim], f32)
            nc.scalar.copy(mb_sb, acc_m)

            # ── per-sample term, 4 rows per column-tiled PE group ──
            wt_sb = singles.tile([bpc, out_dim], f32)
            kB = kF = 0
            t = 0
            for g in range(ngrp):
                acc4 = psg.tile([128, out_dim], f32, tag="acc",
                                name=f"acc{g}")
                stats = [None] * (nib * GRP)
                whs = [None] * (nib * GRP)
                for ib in range(nib):
                    for j in range(GRP):
                        c = ch[t]
                        t += 1
                        if c == "B":
                            w_t = wbf.tile([128, jjb, out_dim], bf16, tag="w",
                                           name=f"w_t{g}_{ib}_{j}")
                            nc.sync.dma_start(
                                out=w_t,
                                in_=W_bf[kB].rearrange(
                                    "(p jj) o -> p jj o", jj=jjb))
                            kB += 1
                            w_h = hpool.tile([128, jjb, out_dim], bf16,
                                             tag="wh", name=f"w_h{g}_{ib}_{j}")
                            for hf in range(2):
                                sl2 = slice(2 * hf, 2 * hf + 2)
                                nc.vector.tensor_mul(
                                    w_h[:, sl2], w_t[:, sl2],
                                    S_sb[:, ib, sl2])
                            stat = xTh_sb
                        else:
                            w8 = wf8.tile([128, jjb, out_dim], fp8e3,
                                          tag="w8", name=f"w8_{g}_{ib}_{j}")
                            nc.sync.dma_start(
                                out=w8,
                                in_=W_f8[kF].rearrange(
                                    "(p jj) o -> p jj o", jj=jjb))
                            kF += 1
                            w_h = hpool.tile([128, jjb, out_dim], bf16,
                                             tag="wh", name=f"w_h{g}_{ib}_{j}")
                            if c == "A":
                                w_c = cpool.tile([128, jjb, out_dim], bf16,
                                                 tag="wc",
                                                 name=f"w_c{g}_{ib}_{j}")
                                nc.scalar.copy(w_c, w8)
                                for hf in range(2):
                                    sl2 = slice(2 * hf, 2 * hf + 2)
                                    nc.vector.tensor_mul(
                                        w_h[:, sl2], w_c[:, sl2],
                                        S_sb[:, ib, sl2])
                            else:  # G
                                nc.gpsimd.tensor_mul(w_h, w8, S_sb[:, ib])
                            stat = xT32_sb
                        stats[ib * GRP + j] = stat
                        whs[ib * GRP + j] = w_h
                    # issue the 4 rows' matmuls j-innermost so they run
                    # concurrently in disjoint 32-column groups
                    for jj in range(jjb):
                        for n in range(nch):
                            for j in range(GRP):
                                b = g * GRP + j
                                nc.tensor.matmul(
                                    acc4[32 * j:32 * j + 1,
                                         n * chunk:(n + 1) * chunk],
                                    stats[ib * GRP + j][:, ib, jj, b:b + 1],
                                    whs[ib * GRP + j][:, jj,
                                                      n * chunk:(n + 1) * chunk],
                                    start=(ib == 0 and jj == 0),
                                    stop=(ib == nib - 1 and jj == jjb - 1),
                                    tile_position=(0, 32 * j),
                                    skip_group_check=True)
                # drain all 4 rows with one ACT copy + one scatter DMA
                col4 = opool.tile([128, out_dim], f32, tag="col",
                                  name=f"col{g}")
                # alternate group drains between ACT and DVE
                if g % 2 == 0:
                    nc.scalar.copy(col4, acc4)
                else:
                    nc.vector.tensor_copy(col4, acc4)
                nc.scalar.dma_start(
                    out=wt_sb[g * GRP:(g + 1) * GRP, :],
                    in_=col4.rearrange("(j q) o -> j q o", j=GRP)[:, 0])

            # ── merge and write out ──
            nc.vector.tensor_add(wt_sb, wt_sb, mb_sb)
            nc.scalar.dma_start(out=out, in_=wt_sb)

    nc.finalize()
    _BUILT[key] = nc
    return nc


def _softplus(x):
    return np.logaddexp(0.0, x.astype(np.float32)).astype(np.float32)


def _run(x, W, mean, log_std, bias, **kwargs):
    import ml_dtypes
    bf16 = ml_dtypes.bfloat16
    fp8 = ml_dtypes.float8_e3m4
    jjb = 4
    ibsz = 128 * jjb
    nib = IN // ibsz
    ngrp = BPC // GRP
    ch = _channels(ngrp * nib)

    x = np.ascontiguousarray(x, dtype=np.float32)
    W = np.ascontiguousarray(W, dtype=np.float32)
    mean_h = np.ascontiguousarray(mean, dtype=np.float32).astype(bf16)
    bias2 = np.ascontiguousarray(bias, dtype=np.float32).reshape(1, OUT)
    S = _softplus(log_std).astype(bf16)

    nc = build_bass()
    in_maps = []
    for cix in range(NCORES):
        sl = slice(cix * BPC, (cix + 1) * BPC)
        Wc = W[sl]  # [BPC, IN, OUT] f32
        bf_tiles, f8_tiles = [], []
        ti = 0
        for g in range(ngrp):
            for ib in range(nib):
                for j in range(GRP):
                    tile_np = Wc[g * GRP + j, ib * ibsz:(ib + 1) * ibsz, :]
                    if ch[ti] == "B":
                        bf_tiles.append(tile_np.astype(bf16))
                    else:
                        f8_tiles.append((tile_np * F8SCALE).astype(fp8))
                    ti += 1
        in_maps.append({
            "xTh": np.ascontiguousarray(x[sl].T).astype(bf16),
            "W_bf": np.stack(bf_tiles),
            "W_f8": np.stack(f8_tiles),
            "S": S,
            "mean": mean_h,
            "bias": bias2,
        })
    res = run_bass_kernel_spmd(nc, in_maps, core_ids=list(range(NCORES)),
                               **kwargs)
    out = np.concatenate([res.results[c]["out"] for c in range(NCORES)],
                         axis=0)
    return out, res


def kernel(x, W, mean, log_std, bias):
    return _run(x, W, mean, log_std, bias)[0]


# revision 19
# speedup vs baseline: 1.6678x; 1.0216x over previous
"""Bayesian dense layer (per-sample reparameterized weights) on 8 TRN2 NeuronCores.

Computes out[b] = x[b] @ (W[b] * softplus(log_std) + mean) + bias for
B=512, IN=OUT=1024, data-parallel over the batch axis (64 rows per core).

v3b: W is uploaded per-tile in a mix of bf16 and fp8e3 (e3m4, pre-scaled x32
on host; the matching stationary x column is pre-divided by 32 on device, an
exact exponent shift).  fp8 tiles reach the bf16 matmul path via ACT
copy-convert (then DVE multiply) or a direct GpSimd fp8xbf16 multiply.
Batch rows are processed in groups of 4 with column-tiled matmuls
(tile_position=(0,32j)): the four rows' N=512 matmuls run concurrently in
disjoint 32-column groups of the PE array, and their PSUM accumulators live
at partitions {0,32,64,96} of one [128, OUT] psum tile, so a group drains
with a single ACT copy + a single 4-row scatter DMA.

Per-core budget at the ~275 us target: HBM ~91 MB @ ~330 GB/s, ACT ~265 us
(converts + drains), GpSimd ~266 us, DVE ~220 us, PE no longer binding.
"""

import os
import sys

for _p in ("/root/.axon_site", "/root/.axon_site/_ro/trn_rl_repo",
           "/root/.axon_site/_ro/pypackages"):
    if os.path.isdir(_p) and _p not in sys.path:
        sys.path.append(_p)

import numpy as np

import concourse.bass as bass
import concourse.mybir as mybir
import concourse.tile as tile
from concourse import bacc
from concourse.bass_utils import run_bass_kernel_spmd

B, IN, OUT = 512, 1024, 1024
NCORES = 8
BPC = B // NCORES  # batch rows per core
F8SCALE = 32.0     # host premultiplies fp8 tiles by this; x column divided
GRP = 4            # batch rows per column-tiled PE group

_BUILT = {}


def _channels(nsets):
    """Channel per tile, consumed in (set, j) order; one set = 4 tiles of
    consecutive rows b=4g+j at one i-block.  GpSimd is unusable here (any GP
    op blocks concurrent DVE tensor_tensor on the shared SBUF port), so all
    multiplies run on DVE; ACT converts as many fp8 tiles as it can absorb."""
    ch = []
    for s in range(nsets):
        if s >= nsets - 2:
            ch += ["B", "B", "B", "B"]   # convert-free tail
        elif s % 4 == 1:
            ch += ["A", "B", "A", "A"]
        else:
            ch += ["A", "B", "A", "B"]
    return ch


def build_bass(bpc=BPC, in_dim=IN, out_dim=OUT):
    key = (bpc, in_dim, out_dim)
    if key in _BUILT:
        return _BUILT[key]

    f32 = mybir.dt.float32
    bf16 = mybir.dt.bfloat16
    fp8e3 = mybir.dt.float8e3
    jjb = 4
    ibsz = 128 * jjb              # 512
    nib = in_dim // ibsz          # 2
    nch = max(1, out_dim // 512)
    chunk = out_dim // nch
    ngrp = bpc // GRP             # 16 row groups
    nset = ngrp * nib             # 32 tile sets of 4

    ch = _channels(nset)
    nBF = sum(c == "B" for c in ch)
    nF8 = len(ch) - nBF

    nc = bacc.Bacc("TRN2", target_bir_lowering=False, debug=False,
                   num_devices=NCORES)

    xTh = nc.dram_tensor("xTh", [in_dim, bpc], bf16, kind="ExternalInput").ap()
    W_bf = nc.dram_tensor("W_bf", [max(nBF, 1), ibsz, out_dim], bf16,
                          kind="ExternalInput").ap()
    W_f8 = nc.dram_tensor("W_f8", [max(nF8, 1), ibsz, out_dim], fp8e3,
                          kind="ExternalInput").ap()
    S = nc.dram_tensor("S", [in_dim, out_dim], bf16, kind="ExternalInput").ap()
    mean = nc.dram_tensor("mean", [in_dim, out_dim], bf16,
                          kind="ExternalInput").ap()
    bias = nc.dram_tensor("bias", [1, out_dim], f32, kind="ExternalInput").ap()
    out = nc.dram_tensor("out", [bpc, out_dim], f32,
                         kind="ExternalOutput").ap()

    with tile.TileContext(nc) as tc:
        with (
            tc.tile_pool(name="singles", bufs=1) as singles,
            tc.tile_pool(name="wbf", bufs=5) as wbf,
            tc.tile_pool(name="wf8", bufs=12) as wf8,
            tc.tile_pool(name="cpool", bufs=3) as cpool,
            tc.tile_pool(name="hpool", bufs=7) as hpool,
            tc.tile_pool(name="opool", bufs=2) as opool,
            tc.tile_pool(name="psum", bufs=1, space="PSUM") as psum,
            tc.tile_pool(name="psg", bufs=2, space="PSUM") as psg,
        ):
            xTh_sb = singles.tile([128, nib, jjb, bpc], bf16)
            nc.sync.dma_start(
                out=xTh_sb,
                in_=xTh.rearrange("(ib p jj) b -> p ib jj b", p=128, jj=jjb))
            xT32_sb = singles.tile([128, nib, jjb, bpc], bf16)
            nc.vector.tensor_scalar_mul(xT32_sb, xTh_sb, 1.0 / F8SCALE)
            S_sb = singles.tile([128, nib, jjb, out_dim], bf16)
            nc.sync.dma_start(
                out=S_sb,
                in_=S.rearrange("(ib p jj) o -> p ib jj o", p=128, jj=jjb))
            bias_sb = singles.tile([1, out_dim], f32)
            nc.sync.dma_start(out=bias_sb, in_=bias)
            ones = singles.tile([1, bpc], f32)
            nc.vector.memset(ones, 1.0)

            # ── per-sample term, 4 rows per column-tiled PE group ──
            wt_sb = singles.tile([bpc, out_dim], f32)
            kB = kF = 0
            t = 0
            for g in range(ngrp):
                acc4 = psg.tile([128, out_dim], f32, tag="acc",
                                name=f"acc{g}")
                stats = [None] * (nib * GRP)
                whs = [None] * (nib * GRP)
                for ib in range(nib):
                    for j in range(GRP):
                        c = ch[t]
                        t += 1
                        if c == "B":
                            w_t = wbf.tile([128, jjb, out_dim], bf16, tag="w",
                                           name=f"w_t{g}_{ib}_{j}")
                            nc.sync.dma_start(
                                out=w_t,
                                in_=W_bf[kB].rearrange(
                                    "(p jj) o -> p jj o", jj=jjb))
                            kB += 1
                            w_h = hpool.tile([128, jjb, out_dim], bf16,
                                             tag="wh", name=f"w_h{g}_{ib}_{j}")
                            for hf in range(2):
                                sl2 = slice(2 * hf, 2 * hf + 2)
                                nc.vector.tensor_mul(
                                    w_h[:, sl2], w_t[:, sl2],
                                    S_sb[:, ib, sl2])
                            stat = xTh_sb
                        else:
                            w8 = wf8.tile([128, jjb, out_dim], fp8e3,
                                          tag="w8", name=f"w8_{g}_{ib}_{j}")
                            nc.sync.dma_start(
                                out=w8,
                                in_=W_f8[kF].rearrange(
                                    "(p jj) o -> p jj o", jj=jjb))
                            kF += 1
                            w_h = hpool.tile([128, jjb, out_dim], bf16,
                                             tag="wh", name=f"w_h{g}_{ib}_{j}")
                            if c == "A":
                                w_c = cpool.tile([128, jjb, out_dim], bf16,
                                                 tag="wc",
                                                 name=f"w_c{g}_{ib}_{j}")
                                nc.scalar.copy(w_c, w8)
                                for hf in range(2):
                                    sl2 = slice(2 * hf, 2 * hf + 2)
                                    nc.vector.tensor_mul(
                                        w_h[:, sl2], w_c[:, sl2],
                                        S_sb[:, ib, sl2])
                            else:  # G
                                nc.gpsimd.tensor_mul(w_h, w8, S_sb[:, ib])
                            stat = xT32_sb
                        stats[ib * GRP + j] = stat
                        whs[ib * GRP + j] = w_h
                    # issue the 4 rows' matmuls j-innermost so they run
                    # concurrently in disjoint 32-column groups
                    for jj in range(jjb):
                        for n in range(nch):
                            for j in range(GRP):
                                b = g * GRP + j
                                nc.tensor.matmul(
                                    acc4[32 * j:32 * j + 1,
                                         n * chunk:(n + 1) * chunk],
                                    stats[ib * GRP + j][:, ib, jj, b:b + 1],
                                    whs[ib * GRP + j][:, jj,
                                                      n * chunk:(n + 1) * chunk],
                                    start=(ib == 0 and jj == 0),
                                    stop=(ib == nib - 1 and jj == jjb - 1),
                                    tile_position=(0, 32 * j),
                                    skip_group_check=True)
                # drain all 4 rows with one ACT copy + one scatter DMA
                col4 = opool.tile([128, out_dim], f32, tag="col",
                                  name=f"col{g}")
                nc.scalar.copy(col4, acc4)
                nc.scalar.dma_start(
                    out=wt_sb[g * GRP:(g + 1) * GRP, :],
                    in_=col4.rearrange("(j q) o -> j q o", j=GRP)[:, 0])

            # ── mean term at full PE width (runs in the pipeline tail when
            # the PE and DMA are otherwise idle): mb = xTh.T @ mean + bias ──
            acc_m = psum.tile([bpc, out_dim], f32)
            for ib in range(nib):
                m_t = wbf.tile([128, jjb, out_dim], bf16, tag="w",
                               name=f"m_t{ib}")
                nc.sync.dma_start(
                    out=m_t,
                    in_=mean[ib * ibsz:(ib + 1) * ibsz, :]
                    .rearrange("(p jj) o -> p jj o", jj=jjb))
                for jj in range(jjb):
                    for n in range(nch):
                        nc.tensor.matmul(
                            acc_m[:, n * chunk:(n + 1) * chunk],
                            xTh_sb[:, ib, jj, :],
                            m_t[:, jj, n * chunk:(n + 1) * chunk],
                            start=(ib == 0 and jj == 0), stop=False,
                            skip_group_check=True)
            for n in range(nch):
                nc.tensor.matmul(
                    acc_m[:, n * chunk:(n + 1) * chunk],
                    ones,
                    bias_sb[:, n * chunk:(n + 1) * chunk],
                    start=False, stop=True, skip_group_check=True)
            mb_sb = singles.tile([bpc, out_dim], f32)
            nc.scalar.copy(mb_sb, acc_m)

            # ── merge and write out ──
            nc.vector.tensor_add(wt_sb, wt_sb, mb_sb)
            nc.scalar.dma_start(out=out, in_=wt_sb)

    nc.finalize()
    _BUILT[key] = nc
    return nc


def _softplus(x):
    return np.logaddexp(0.0, x.astype(np.float32)).astype(np.float32)


def _run(x, W, mean, log_std, bias, **kwargs):
    import ml_dtypes
    bf16 = ml_dtypes.bfloat16
    fp8 = ml_dtypes.float8_e3m4
    jjb = 4
    ibsz = 128 * jjb
    nib = IN // ibsz
    ngrp = BPC // GRP
    ch = _channels(ngrp * nib)

    x = np.ascontiguousarray(x, dtype=np.float32)
    W = np.ascontiguousarray(W, dtype=np.float32)
    mean_h = np.ascontiguousarray(mean, dtype=np.float32).astype(bf16)
    bias2 = np.ascontiguousarray(bias, dtype=np.float32).reshape(1, OUT)
    S = _softplus(log_std).astype(bf16)

    nc = build_bass()
    in_maps = []
    for cix in range(NCORES):
        sl = slice(cix * BPC, (cix + 1) * BPC)
        Wc = W[sl]  # [BPC, IN, OUT] f32
        bf_tiles, f8_tiles = [], []
        ti = 0
        for g in range(ngrp):
            for ib in range(nib):
                for j in range(GRP):
                    tile_np = Wc[g * GRP + j, ib * ibsz:(ib + 1) * ibsz, :]
                    if ch[ti] == "B":
                        bf_tiles.append(tile_np.astype(bf16))
                    else:
                        f8_tiles.append((tile_np * F8SCALE).astype(fp8))
                    ti += 1
        in_maps.append({
            "xTh": np.ascontiguousarray(x[sl].T).astype(bf16),
            "W_bf": np.stack(bf_tiles),
            "W_f8": np.stack(f8_tiles),
            "S": S,
            "mean": mean_h,
            "bias": bias2,
        })
    res = run_bass_kernel_spmd(nc, in_maps, core_ids=list(range(NCORES)),
                               **kwargs)
    out = np.concatenate([res.results[c]["out"] for c in range(NCORES)],
                         axis=0)
    return out, res


def kernel(x, W, mean, log_std, bias):
    return _run(x, W, mean, log_std, bias)[0]


# revision 22
# speedup vs baseline: 1.6799x; 1.0073x over previous
"""Bayesian dense layer (per-sample reparameterized weights) on 8 TRN2 NeuronCores.

Computes out[b] = x[b] @ (W[b] * softplus(log_std) + mean) + bias for
B=512, IN=OUT=1024, data-parallel over the batch axis (64 rows per core).

v3b: W is uploaded per-tile in a mix of bf16 and fp8e3 (e3m4, pre-scaled x32
on host; the matching stationary x column is pre-divided by 32 on device, an
exact exponent shift).  fp8 tiles reach the bf16 matmul path via ACT
copy-convert (then DVE multiply) or a direct GpSimd fp8xbf16 multiply.
Batch rows are processed in groups of 4 with column-tiled matmuls
(tile_position=(0,32j)): the four rows' N=512 matmuls run concurrently in
disjoint 32-column groups of the PE array, and their PSUM accumulators live
at partitions {0,32,64,96} of one [128, OUT] psum tile, so a group drains
with a single ACT copy + a single 4-row scatter DMA.

Per-core budget at the ~275 us target: HBM ~91 MB @ ~330 GB/s, ACT ~265 us
(converts + drains), GpSimd ~266 us, DVE ~220 us, PE no longer binding.
"""

import os
import sys

for _p in ("/root/.axon_site", "/root/.axon_site/_ro/trn_rl_repo",
           "/root/.axon_site/_ro/pypackages"):
    if os.path.isdir(_p) and _p not in sys.path:
        sys.path.append(_p)

import numpy as np

import concourse.bass as bass
import concourse.mybir as mybir
import concourse.tile as tile
from concourse import bacc
from concourse.bass_utils import run_bass_kernel_spmd

B, IN, OUT = 512, 1024, 1024
NCORES = 8
BPC = B // NCORES  # batch rows per core
F8SCALE = 32.0     # host premultiplies fp8 tiles by this; x column divided
GRP = 4            # batch rows per column-tiled PE group

_BUILT = {}


def _channels(nsets):
    """Channel per tile, consumed in (set, j) order; one set = 4 tiles of
    consecutive rows b=4g+j at one i-block.  GpSimd is unusable here (any GP
    op blocks concurrent DVE tensor_tensor on the shared SBUF port), so all
    multiplies run on DVE; ACT converts as many fp8 tiles as it can absorb."""
    ch = []
    for s in range(nsets):
        if s >= nsets - 2:
            ch += ["B", "B", "B", "B"]   # convert-free tail
        elif s % 4 == 1:
            ch += ["A", "B", "A", "A"]
        else:
            ch += ["A", "B", "A", "B"]
    return ch


def build_bass(bpc=BPC, in_dim=IN, out_dim=OUT):
    key = (bpc, in_dim, out_dim)
    if key in _BUILT:
        return _BUILT[key]

    f32 = mybir.dt.float32
    bf16 = mybir.dt.bfloat16
    fp8e3 = mybir.dt.float8e3
    jjb = 4
    ibsz = 128 * jjb              # 512
    nib = in_dim // ibsz          # 2
    nch = max(1, out_dim // 512)
    chunk = out_dim // nch
    ngrp = bpc // GRP             # 16 row groups
    nset = ngrp * nib             # 32 tile sets of 4

    ch = _channels(nset)
    nBF = sum(c == "B" for c in ch)
    nF8 = len(ch) - nBF

    nc = bacc.Bacc("TRN2", target_bir_lowering=False, debug=False,
                   num_devices=NCORES)

    xTh = nc.dram_tensor("xTh", [in_dim, bpc], bf16, kind="ExternalInput").ap()
    W_bf = nc.dram_tensor("W_bf", [max(nBF, 1), ibsz, out_dim], bf16,
                          kind="ExternalInput").ap()
    W_f8 = nc.dram_tensor("W_f8", [max(nF8, 1), ibsz, out_dim], fp8e3,
                          kind="ExternalInput").ap()
    S = nc.dram_tensor("S", [in_dim, out_dim], bf16, kind="ExternalInput").ap()
    mean = nc.dram_tensor("mean", [in_dim, out_dim], bf16,
                          kind="ExternalInput").ap()
    bias = nc.dram_tensor("bias", [1, out_dim], f32, kind="ExternalInput").ap()
    out = nc.dram_tensor("out", [bpc, out_dim], f32,
                         kind="ExternalOutput").ap()

    with tile.TileContext(nc) as tc:
        with (
            tc.tile_pool(name="singles", bufs=1) as singles,
            tc.tile_pool(name="wbf", bufs=5) as wbf,
            tc.tile_pool(name="wf8", bufs=12) as wf8,
            tc.tile_pool(name="cpool", bufs=3) as cpool,
            tc.tile_pool(name="hpool", bufs=7) as hpool,
            tc.tile_pool(name="opool", bufs=2) as opool,
            tc.tile_pool(name="psum", bufs=1, space="PSUM") as psum,
            tc.tile_pool(name="psg", bufs=2, space="PSUM") as psg,
        ):
            # singles ride the scalar HWDGE ring so W tiles start immediately
            # on the sync ring
            xTh_sb = singles.tile([128, nib, jjb, bpc], bf16)
            nc.scalar.dma_start(
                out=xTh_sb,
                in_=xTh.rearrange("(ib p jj) b -> p ib jj b", p=128, jj=jjb))
            xT32_sb = singles.tile([128, nib, jjb, bpc], bf16)
            nc.vector.tensor_scalar_mul(xT32_sb, xTh_sb, 1.0 / F8SCALE)
            S_sb = singles.tile([128, nib, jjb, out_dim], bf16)
            nc.scalar.dma_start(
                out=S_sb,
                in_=S.rearrange("(ib p jj) o -> p ib jj o", p=128, jj=jjb))
            bias_sb = singles.tile([1, out_dim], f32)
            nc.scalar.dma_start(out=bias_sb, in_=bias)
            ones = singles.tile([1, bpc], f32)
            nc.vector.memset(ones, 1.0)

            # mean term at full PE width: mb = xTh.T @ mean + bias.  Emitted
            # two groups before the end so its DMA and matmuls hide inside
            # the pipeline instead of serializing the tail.
            mb_sb = singles.tile([bpc, out_dim], f32)

            def emit_mean_term():
                acc_m = psum.tile([bpc, out_dim], f32)
                for ib in range(nib):
                    m_t = wbf.tile([128, jjb, out_dim], bf16, tag="w",
                                   name=f"m_t{ib}")
                    nc.scalar.dma_start(
                        out=m_t,
                        in_=mean[ib * ibsz:(ib + 1) * ibsz, :]
                        .rearrange("(p jj) o -> p jj o", jj=jjb))
                    for jj in range(jjb):
                        for n in range(nch):
                            nc.tensor.matmul(
                                acc_m[:, n * chunk:(n + 1) * chunk],
                                xTh_sb[:, ib, jj, :],
                                m_t[:, jj, n * chunk:(n + 1) * chunk],
                                start=(ib == 0 and jj == 0), stop=False,
                                skip_group_check=True)
                for n in range(nch):
                    nc.tensor.matmul(
                        acc_m[:, n * chunk:(n + 1) * chunk],
                        ones,
                        bias_sb[:, n * chunk:(n + 1) * chunk],
                        start=False, stop=True, skip_group_check=True)
                nc.scalar.copy(mb_sb, acc_m)

            # ── per-sample term, 4 rows per column-tiled PE group ──
            wt_sb = singles.tile([bpc, out_dim], f32)
            kB = kF = 0
            t = 0
            for g in range(ngrp):
                if g == ngrp - 2:
                    emit_mean_term()
                acc4 = psg.tile([128, out_dim], f32, tag="acc",
                                name=f"acc{g}")
                stats = [None] * (nib * GRP)
                whs = [None] * (nib * GRP)
                for ib in range(nib):
                    for j in range(GRP):
                        c = ch[t]
                        t += 1
                        if c == "B":
                            w_t = wbf.tile([128, jjb, out_dim], bf16, tag="w",
                                           name=f"w_t{g}_{ib}_{j}")
                            nc.sync.dma_start(
                                out=w_t,
                                in_=W_bf[kB].rearrange(
                                    "(p jj) o -> p jj o", jj=jjb))
                            kB += 1
                            w_h = hpool.tile([128, jjb, out_dim], bf16,
                                             tag="wh", name=f"w_h{g}_{ib}_{j}")
                            for hf in range(2):
                                sl2 = slice(2 * hf, 2 * hf + 2)
                                nc.vector.tensor_mul(
                                    w_h[:, sl2], w_t[:, sl2],
                                    S_sb[:, ib, sl2])
                            stat = xTh_sb
                        else:
                            w8 = wf8.tile([128, jjb, out_dim], fp8e3,
                                          tag="w8", name=f"w8_{g}_{ib}_{j}")
                            nc.sync.dma_start(
                                out=w8,
                                in_=W_f8[kF].rearrange(
                                    "(p jj) o -> p jj o", jj=jjb))
                            kF += 1
                            w_h = hpool.tile([128, jjb, out_dim], bf16,
                                             tag="wh", name=f"w_h{g}_{ib}_{j}")
                            if c == "A":
                                w_c = cpool.tile([128, jjb, out_dim], bf16,
                                                 tag="wc",
                                                 name=f"w_c{g}_{ib}_{j}")
                                nc.scalar.copy(w_c, w8)
                                for hf in range(2):
                                    sl2 = slice(2 * hf, 2 * hf + 2)
                                    nc.vector.tensor_mul(
                                        w_h[:, sl2], w_c[:, sl2],
                                        S_sb[:, ib, sl2])
                            else:  # G
                                nc.gpsimd.tensor_mul(w_h, w8, S_sb[:, ib])
                            stat = xT32_sb
                        stats[ib * GRP + j] = stat
                        whs[ib * GRP + j] = w_h
                    # issue the 4 rows' matmuls j-innermost so they run
                    # concurrently in disjoint 32-column groups
                    for jj in range(jjb):
                        for n in range(nch):
                            for j in range(GRP):
                                b = g * GRP + j
                                nc.tensor.matmul(
                                    acc4[32 * j:32 * j + 1,
                                         n * chunk:(n + 1) * chunk],
                                    stats[ib * GRP + j][:, ib, jj, b:b + 1],
                                    whs[ib * GRP + j][:, jj,
                                                      n * chunk:(n + 1) * chunk],
                                    start=(ib == 0 and jj == 0),
                                    stop=(ib == nib - 1 and jj == jjb - 1),
                                    tile_position=(0, 32 * j),
                                    skip_group_check=True)
                # drain all 4 rows with one ACT copy + one scatter DMA
                col4 = opool.tile([128, out_dim], f32, tag="col",
                                  name=f"col{g}")
                nc.scalar.copy(col4, acc4)
                nc.scalar.dma_start(
                    out=wt_sb[g * GRP:(g + 1) * GRP, :],
                    in_=col4.rearrange("(j q) o -> j q o", j=GRP)[:, 0])

            # ── merge and write out ──
            nc.vector.tensor_add(wt_sb, wt_sb, mb_sb)
            nc.scalar.dma_start(out=out, in_=wt_sb)

    nc.finalize()
    _BUILT[key] = nc
    return nc


def _softplus(x):
    return np.logaddexp(0.0, x.astype(np.float32)).astype(np.float32)


def _run(x, W, mean, log_std, bias, **kwargs):
    import ml_dtypes
    bf16 = ml_dtypes.bfloat16
    fp8 = ml_dtypes.float8_e3m4
    jjb = 4
    ibsz = 128 * jjb
    nib = IN // ibsz
    ngrp = BPC // GRP
    ch = _channels(ngrp * nib)

    x = np.ascontiguousarray(x, dtype=np.float32)
    W = np.ascontiguousarray(W, dtype=np.float32)
    mean_h = np.ascontiguousarray(mean, dtype=np.float32).astype(bf16)
    bias2 = np.ascontiguousarray(bias, dtype=np.float32).reshape(1, OUT)
    S = _softplus(log_std).astype(bf16)

    nc = build_bass()
    in_maps = []
    for cix in range(NCORES):
        sl = slice(cix * BPC, (cix + 1) * BPC)
        Wc = W[sl]  # [BPC, IN, OUT] f32
        bf_tiles, f8_tiles = [], []
        ti = 0
        for g in range(ngrp):
            for ib in range(nib):
                for j in range(GRP):
                    tile_np = Wc[g * GRP + j, ib * ibsz:(ib + 1) * ibsz, :]
                    if ch[ti] == "B":
                        bf_tiles.append(tile_np.astype(bf16))
                    else:
                        f8_tiles.append((tile_np * F8SCALE).astype(fp8))
                    ti += 1
        in_maps.append({
            "xTh": np.ascontiguousarray(x[sl].T).astype(bf16),
            "W_bf": np.stack(bf_tiles),
            "W_f8": np.stack(f8_tiles),
            "S": S,
            "mean": mean_h,
            "bias": bias2,
        })
    res = run_bass_kernel_spmd(nc, in_maps, core_ids=list(range(NCORES)),
                               **kwargs)
    out = np.concatenate([res.results[c]["out"] for c in range(NCORES)],
                         axis=0)
    return out, res


def kernel(x, W, mean, log_std, bias):
    return _run(x, W, mean, log_std, bias)[0]
